# revision 17
# baseline (speedup 1.0000x reference)
"""DeeperHNN hypergraph message passing kernel for 8 Trainium2 NeuronCores.

Strategy: nodes (and incidence entries, partitioned by vertex) sharded across
8 cores; theta is applied in EDGE space (it commutes with the segment-sum):

  per layer:
    Phase A: partial Yh[e] = de_inv[e] * sum_{local v in e} t[v]
             (dma_gather t rows by token -> host-precomputed one-hot S matmul)
    ReduceScatter(Yh) -> each core owns 20 edge blocks
    theta:   Ye_c = Yh_c @ thetaW[i] + 1*thetaB[i]   (on owned 1/8 slice)
    AllGather(Ye_c) -> YeF replicated
    Phase B: conv[v] = relu(dv_inv[v] * sum_{e: v in e} YeF[e])
             h' = h + conv (residual, layers 1-3); tail t = relu(LN(h'))

Local node ids are relabeled (deal-by-degree) so phase-B blocks have ~equal
token counts (exactly 8 slots per block).  All segment structure (token index
tables, one-hot S matrices, de/dv scale columns) is precomputed on host.
"""

import numpy as np

import concourse.bacc as bacc
import concourse.bass as bass
import concourse.mybir as mybir
import concourse.tile as tile
from concourse.bass_utils import run_bass_kernel_spmd
from concourse.masks import make_identity

import ml_dtypes

P = 128
BF16_NP = ml_dtypes.bfloat16
F32 = mybir.dt.float32
BF16 = mybir.dt.bfloat16
I16 = mybir.dt.int16
I32 = mybir.dt.int32
AF = mybir.ActivationFunctionType
ALU = mybir.AluOpType


def _cdiv(a, b):
    return (a + b - 1) // b


class Prep:
    pass


def host_prep(vidx, eidx, N, E, C, gmax=24):
    """Build the static segment/gather structure shared by the SPMD program."""
    p = Prep()
    NP = N // C
    NBV = _cdiv(NP, P)
    NPAD = NBV * P
    NBE = _cdiv(E, P)            # 157 blocks carry real edges
    NBE_pad = _cdiv(NBE, C) * C  # 160 blocks so RS slices are whole blocks
    EPAD = NBE_pad * P
    p.N, p.E, p.C, p.NP = N, E, C, NP
    p.NBV, p.NPAD, p.NBE, p.NBE_pad, p.EPAD = NBV, NPAD, NBE, NBE_pad, EPAD

    vidx = np.asarray(vidx).astype(np.int64)
    eidx = np.asarray(eidx).astype(np.int64)
    de = np.bincount(eidx, minlength=E)
    de_inv = (1.0 / np.maximum(de, 1.0)).astype(np.float32)
    core = vidx // NP

    # ---- per-core node relabeling: deal by degree so block loads equalize ----
    # new local id of old local node o on core c: perm[c][o]
    perm = np.zeros((C, NP), np.int64)
    for c in range(C):
        lv = vidx[core == c] - c * NP
        dv_loc = np.bincount(lv, minlength=NP)
        order = np.argsort(-dv_loc, kind="stable")  # ranks -> old ids
        r = np.arange(NP)
        rnd, pos = r // NBV, r % NBV
        blk = np.where(rnd % 2 == 0, pos, NBV - 1 - pos)  # snake deal
        perm[c, order] = blk * P + rnd
    p.perm = perm

    dv_full = np.bincount(vidx, minlength=N)
    dv_inv = (1.0 / np.maximum(dv_full, 1.0)).astype(np.float32)
    dvc = np.zeros((C, P, NBV), np.float32)
    for c in range(C):
        vals = np.zeros(NPAD, np.float32)
        vals[perm[c]] = dv_inv[c * NP:(c + 1) * NP]
        dvc[c] = vals.reshape(NBV, P).T
    p.dvc = dvc

    # de_inv per edge-block column [128, NBE_pad] (same on all cores)
    dec = np.zeros(EPAD, np.float32)
    dec[:E] = de_inv
    p.dec = dec.reshape(NBE_pad, P).T.copy()

    # ---- phase A: tokens sorted by edge, grouped into 128-edge blocks ----
    cntA = np.zeros((C, NBE), np.int64)
    A_ev, A_lv = [], []
    for c in range(C):
        m = core == c
        ev = eidx[m]
        lv = perm[c][vidx[m] - c * NP]
        o = np.argsort(ev, kind="stable")
        ev, lv = ev[o], lv[o]
        cntA[c] = np.bincount(ev // P, minlength=NBE)
        A_ev.append(ev)
        A_lv.append(lv)
    slotsA = np.maximum(1, _cdiv(cntA.max(0), P)).astype(np.int64)
    SA = int(slotsA.sum())
    TA = SA * P
    blkslotA = np.zeros(NBE + 1, np.int64)
    np.cumsum(slotsA, out=blkslotA[1:])

    idxA = np.full((C, TA), NPAD, np.int16)     # dummy -> zero row of t
    rA = np.zeros((C, TA), np.float32)
    for c in range(C):
        ev, lv = A_ev[c], A_lv[c]
        blk = ev // P
        starts = np.searchsorted(ev, np.arange(NBE) * P)
        tok = blkslotA[blk] * P + (np.arange(len(ev)) - starts[blk])
        idxA[c, tok] = lv
        rA[c, tok] = ev - blk * P

    # ---- phase B: tokens sorted by (relabeled) vertex ----
    cntB = np.zeros((C, NBV), np.int64)
    B_ee, B_lv = [], []
    for c in range(C):
        m = core == c
        lv = perm[c][vidx[m] - c * NP]
        ee = eidx[m]
        o = np.argsort(lv, kind="stable")
        lv, ee = lv[o], ee[o]
        cntB[c] = np.bincount(lv // P, minlength=NBV)
        B_ee.append(ee)
        B_lv.append(lv)
    slotsB = np.maximum(1, _cdiv(cntB.max(0), P)).astype(np.int64)
    SB = int(slotsB.sum())
    TB = SB * P
    blkslotB = np.zeros(NBV + 1, np.int64)
    np.cumsum(slotsB, out=blkslotB[1:])

    idxB = np.full((C, TB), EPAD, np.int16)     # dummy -> zero row of YeF
    rB = np.zeros((C, TB), np.float32)
    for c in range(C):
        ee, lv = B_ee[c], B_lv[c]
        blk = lv // P
        starts = np.searchsorted(lv, np.arange(NBV) * P)
        tok = blkslotB[blk] * P + (np.arange(len(lv)) - starts[blk])
        idxB[c, tok] = ee
        rB[c, tok] = lv - blk * P

    p.slotsA, p.slotsB, p.SA, p.SB, p.TA, p.TB = slotsA, slotsB, SA, SB, TA, TB
    p.blkslotA, p.blkslotB = blkslotA, blkslotB
    p.idxA_w = np.ascontiguousarray(
        np.tile(idxA.reshape(C, TA // 16, 16).transpose(0, 2, 1), (1, 8, 1)))
    p.idxB_w = np.ascontiguousarray(
        np.tile(idxB.reshape(C, TB // 16, 16).transpose(0, 2, 1), (1, 8, 1)))
    p.rA_m = np.ascontiguousarray(rA.reshape(C, SA, P).transpose(0, 2, 1)).astype(BF16_NP)
    p.rB_m = np.ascontiguousarray(rB.reshape(C, SB, P).transpose(0, 2, 1)).astype(BF16_NP)

    def make_groups(slots, cap):
        groups = []  # (block0, nblocks, slot0, gslots)
        b, nb = 0, len(slots)
        while b < nb:
            s0 = int(slots[:b].sum())
            g = n = 0
            while b + n < nb and g + slots[b + n] <= cap:
                g += int(slots[b + n])
                n += 1
            assert n > 0
            groups.append((b, n, s0, g))
            b += n
        return groups

    p.gmax = gmax
    p.groupsA = make_groups(slotsA, gmax)
    p.groupsB = make_groups(slotsB, gmax)
    return p


# ----------------------------------------------------------------------------
# Device program
# ----------------------------------------------------------------------------
def build_program(p, IN_DIM, H, OUT, L, enable_asserts=False):
    C, NBV, NPAD, NBE, NBE_pad, EPAD = p.C, p.NBV, p.NPAD, p.NBE, p.NBE_pad, p.EPAD
    KI = IN_DIM // P  # 3
    KH = H // P       # 2
    OBLK = NBE_pad // C  # owned edge blocks per core (20)
    GMAX = p.gmax
    GCH = 8              # slots per dma_gather instruction (1024 idx)

    nc = bacc.Bacc(
        "TRN2",
        target_bir_lowering=False,
        debug=False,
        enable_asserts=enable_asserts,
        num_devices=C,
        num_swdge_queues=4,
    )

    # ---- I/O ----
    xT_d = nc.dram_tensor("xT", [IN_DIM, NPAD], BF16, kind="ExternalInput")
    encW_d = nc.dram_tensor("encW", [IN_DIM, H], F32, kind="ExternalInput")
    encB_d = nc.dram_tensor("encB", [H], F32, kind="ExternalInput")
    thW_d = nc.dram_tensor("thW", [L, H, H], F32, kind="ExternalInput")
    thB_d = nc.dram_tensor("thB", [L, H], F32, kind="ExternalInput")
    lnG_d = nc.dram_tensor("lnG", [L, H], F32, kind="ExternalInput")
    lnB_d = nc.dram_tensor("lnB", [L, H], F32, kind="ExternalInput")
    linW_d = nc.dram_tensor("linW", [H, OUT], F32, kind="ExternalInput")
    linB_d = nc.dram_tensor("linB", [OUT], F32, kind="ExternalInput")
    idxA_d = nc.dram_tensor("idxA", [P, p.TA // 16], I16, kind="ExternalInput")
    rA_d = nc.dram_tensor("rA", [P, p.SA], BF16, kind="ExternalInput")
    idxB_d = nc.dram_tensor("idxB", [P, p.TB // 16], I16, kind="ExternalInput")
    rB_d = nc.dram_tensor("rB", [P, p.SB], BF16, kind="ExternalInput")
    dv_d = nc.dram_tensor("dvc", [P, NBV], F32, kind="ExternalInput")
    dec_d = nc.dram_tensor("dec", [P, NBE_pad], F32, kind="ExternalInput")
    out_d = nc.dram_tensor("out", [NPAD, OUT], F32, kind="ExternalOutput")

    # ---- internals ----
    t_d = nc.dram_tensor("t_t", [NPAD + P, H], BF16)     # row NPAD+ is zero dummy
    h_d = nc.dram_tensor("h_t", [NPAD, H], F32)
    YhP_d = nc.dram_tensor("YhP", [EPAD, H], BF16)       # theta'd phase-A partials
    YeF_d = nc.dram_tensor("YeF", [EPAD + P, H], BF16, addr_space="Shared")

    xT_v = xT_d.ap().rearrange("(k q) n -> q k n", q=P)

    from contextlib import ExitStack
    with tile.TileContext(nc) as tc, ExitStack() as es:
        const = es.enter_context(tc.tile_pool(name="const", bufs=1))
        meta = es.enter_context(tc.tile_pool(name="meta", bufs=1))
        gpool = es.enter_context(tc.tile_pool(name="gpool", bufs=2))
        spool = es.enter_context(tc.tile_pool(name="spool", bufs=2))
        wrk = es.enter_context(tc.tile_pool(name="wrk", bufs=3))
        stat = es.enter_context(tc.tile_pool(name="stat", bufs=4))
        opool = es.enter_context(tc.tile_pool(name="opool", bufs=3))
        psA = es.enter_context(tc.tile_pool(name="psA", bufs=2, space="PSUM"))
        psT = es.enter_context(tc.tile_pool(name="psT", bufs=2, space="PSUM"))

        # ---- constants ----
        iota_i = const.tile([P, GMAX, P], I32)
        nc.gpsimd.iota(iota_i[:, :, :], pattern=[[0, GMAX], [1, P]], base=0,
                       channel_multiplier=0)
        iota_f = const.tile([P, GMAX, P], BF16)
        nc.vector.tensor_copy(iota_f[:, :, :], iota_i[:, :, :])
        identf = const.tile([P, P], F32)
        make_identity(nc, identf[:, :])
        ident = const.tile([P, P], BF16)
        nc.vector.tensor_copy(ident[:, :], identf[:, :])
        ones1 = const.tile([1, P], BF16)
        nc.vector.memset(ones1[:, :], 1.0)
        ones8 = const.tile([1, P], BF16)
        nc.vector.memset(ones8[:, :], 1.0 / C)
        epsc = const.tile([P, 1], F32)
        nc.vector.memset(epsc[:, :], 1e-5)

        # weights (loaded f32 via HWDGE, cast to bf16 on DVE)
        def wtile(shape, src_ap, tag):
            f = const.tile(shape, F32, tag=tag + "f")
            nc.sync.dma_start(f[(slice(None),) * len(shape)], src_ap)
            b = const.tile(shape, BF16, tag=tag)
            nc.vector.tensor_copy(b[(slice(None),) * len(shape)],
                                  f[(slice(None),) * len(shape)])
            return b

        encW_t = [wtile([P, H], encW_d[k * P:(k + 1) * P, :], f"encW{k}")
                  for k in range(KI)]
        encB_t = wtile([1, H], encB_d[None, :], "encB")
        thW_t = [[wtile([P, H], thW_d[i, k * P:(k + 1) * P, :], f"thW{i}{k}")
                  for k in range(KH)] for i in range(L)]
        thB_t = [wtile([1, H], thB_d[i:i + 1, :], f"thB{i}") for i in range(L)]
        linW_t = [wtile([P, OUT], linW_d[k * P:(k + 1) * P, :], f"linW{k}")
                  for k in range(KH)]
        linB_t = wtile([1, OUT], linB_d[None, :], "linB")
        lnG_t, lnB_t = [], []
        for i in range(L):
            g = const.tile([P, H], F32, tag=f"lnG{i}")
            b = const.tile([P, H], F32, tag=f"lnB{i}")
            nc.sync.dma_start(g[:, :], lnG_d[i:i + 1, :].partition_broadcast(P).squeeze(1))
            nc.sync.dma_start(b[:, :], lnB_d[i:i + 1, :].partition_broadcast(P).squeeze(1))
            lnG_t.append(g)
            lnB_t.append(b)

        # metadata
        idxA_t = meta.tile([P, p.TA // 16], I16)
        nc.sync.dma_start(idxA_t[:, :], idxA_d[:, :])
        rA_t = meta.tile([P, p.SA], BF16)
        nc.sync.dma_start(rA_t[:, :], rA_d[:, :])
        idxB_t = meta.tile([P, p.TB // 16], I16)
        nc.sync.dma_start(idxB_t[:, :], idxB_d[:, :])
        rB_t = meta.tile([P, p.SB], BF16)
        nc.sync.dma_start(rB_t[:, :], rB_d[:, :])
        dec_t = meta.tile([P, NBE_pad], F32)
        nc.sync.dma_start(dec_t[:, :], dec_d[:, :])
        dv_t = meta.tile([P, NBV], F32)
        nc.sync.dma_start(dv_t[:, :], dv_d[:, :])

        # zero pad zones: t dummy row block, YeF dummy block, YhP pad blocks
        zblk = const.tile([P, H], BF16)
        nc.vector.memset(zblk[:, :], 0.0)
        nc.sync.dma_start(t_d[NPAD:NPAD + P, :], zblk[:, :])
        nc.sync.dma_start(YeF_d[EPAD:EPAD + P, :], zblk[:, :])
        for eb in range(NBE, NBE_pad):
            nc.sync.dma_start(YhP_d[eb * P:(eb + 1) * P, :], zblk[:, :])

        # ------------------------------------------------------------------
        # Encoder: t0 = x @ encW + encB (row-major node blocks)
        # ------------------------------------------------------------------
        for rb in range(NBV):
            xc = wrk.tile([P, KI, P], BF16, tag="xc")
            nc.sync.dma_start(xc[:, :, :], xT_v[:, :, rb * P:(rb + 1) * P])
            ps = psA.tile([P, H], F32, tag="ps256")
            for k in range(KI):
                nc.tensor.matmul(ps[:, :], lhsT=xc[:, k, :], rhs=encW_t[k][:, :],
                                 start=(k == 0), stop=False)
            nc.tensor.matmul(ps[:, :], lhsT=ones1[:1, :], rhs=encB_t[:1, :],
                             start=False, stop=True)
            ob = opool.tile([P, H], BF16, tag="encout")
            nc.scalar.activation(ob[:, :], ps[:, :], AF.Copy)
            nc.sync.dma_start(t_d[rb * P:(rb + 1) * P, :], ob[:, :])

        # ------------------------------------------------------------------
        # Conv layers
        # ------------------------------------------------------------------
        qn = 0
        for li in range(L):
            # ---- Phase A: partial Yh (de_inv * segment-sum of t rows) ----
            for (b0, nb, s0, gs) in p.groupsA:
                G = gpool.tile([P, GMAX, H], BF16, tag="G")
                g0 = 0
                while g0 < gs:
                    gc = min(GCH, gs - g0)
                    nc.gpsimd.dma_gather(
                        out_ap=G[:, g0:g0 + gc, :],
                        in_ap=t_d[:, :],
                        idxs_ap=idxA_t[:, (s0 + g0) * 8:(s0 + g0 + gc) * 8],
                        num_idxs=gc * P,
                        num_idxs_reg=gc * P,
                        elem_size=H,
                        queue_num=qn,
                    )
                    qn = (qn + 1) % 4
                    g0 += gc
                S = spool.tile([P, GMAX, P], BF16, tag="S")
                rb_ap = rA_t[:, s0:s0 + gs].unsqueeze(2).broadcast_to([P, gs, P])
                nc.vector.tensor_tensor(S[:, :gs, :], iota_f[:, :gs, :], rb_ap,
                                        op=ALU.is_equal)
                for bi in range(nb):
                    eb = b0 + bi
                    ls = int(p.blkslotA[eb]) - s0
                    sb = int(p.slotsA[eb])
                    ps = psA.tile([P, H], F32, tag="ps256")
                    for s in range(ls, ls + sb):
                        nc.tensor.matmul(ps[:, :], lhsT=S[:, s, :], rhs=G[:, s, :],
                                         start=(s == ls), stop=(s == ls + sb - 1))
                    yb = opool.tile([P, H], BF16, tag="yhb")
                    nc.scalar.activation(yb[:, :], ps[:, :], AF.Copy,
                                         scale=dec_t[:, eb:eb + 1])
                    # theta on the partial (linear: sums commute with theta);
                    # each core adds thB/C so the AllReduce yields +thB once
                    yT = opool.tile([P, KH, P], BF16, tag="yT")
                    for k in range(KH):
                        pst = psT.tile([P, P], F32, tag="psT")
                        nc.tensor.matmul(pst[:, :], lhsT=yb[:, k * P:(k + 1) * P],
                                         rhs=ident[:, :], start=True, stop=True)
                        nc.scalar.activation(yT[:, k, :], pst[:, :], AF.Copy)
                    ps2 = psA.tile([P, H], F32, tag="ps256b")
                    for k in range(KH):
                        nc.tensor.matmul(ps2[:, :], lhsT=yT[:, k, :],
                                         rhs=thW_t[li][k][:, :],
                                         start=(k == 0), stop=False)
                    nc.tensor.matmul(ps2[:, :], lhsT=ones8[:1, :], rhs=thB_t[li][:1, :],
                                     start=False, stop=True)
                    yeb = opool.tile([P, H], BF16, tag="yeb")
                    nc.scalar.activation(yeb[:, :], ps2[:, :], AF.Copy)
                    nc.sync.dma_start(YhP_d[eb * P:(eb + 1) * P, :], yeb[:, :])

            # ---- AllReduce theta'd edge partials ----
            nc.gpsimd.collective_compute(
                "AllReduce", ALU.add,
                replica_groups=[list(range(C))],
                ins=[YhP_d.ap()[:EPAD, :]],
                outs=[YeF_d.ap()[:EPAD, :]],
            )

            # ---- Phase B: conv + residual + LN tail ----
            lnxt = li + 1 if li + 1 < L else 0
            for (b0, nb, s0, gs) in p.groupsB:
                G = gpool.tile([P, GMAX, H], BF16, tag="G")
                g0 = 0
                while g0 < gs:
                    gc = min(GCH, gs - g0)
                    nc.gpsimd.dma_gather(
                        out_ap=G[:, g0:g0 + gc, :],
                        in_ap=YeF_d[:, :],
                        idxs_ap=idxB_t[:, (s0 + g0) * 8:(s0 + g0 + gc) * 8],
                        num_idxs=gc * P,
                        num_idxs_reg=gc * P,
                        elem_size=H,
                        queue_num=qn,
                    )
                    qn = (qn + 1) % 4
                    g0 += gc
                S = spool.tile([P, GMAX, P], BF16, tag="S")
                rb_ap = rB_t[:, s0:s0 + gs].unsqueeze(2).broadcast_to([P, gs, P])
                nc.vector.tensor_tensor(S[:, :gs, :], iota_f[:, :gs, :], rb_ap,
                                        op=ALU.is_equal)
                for bi in range(nb):
                    vb = b0 + bi
                    ls = int(p.blkslotB[vb]) - s0
                    sb = int(p.slotsB[vb])
                    ps = psA.tile([P, H], F32, tag="ps256")
                    for s in range(ls, ls + sb):
                        nc.tensor.matmul(ps[:, :], lhsT=S[:, s, :], rhs=G[:, s, :],
                                         start=(s == ls), stop=(s == ls + sb - 1))
                    # relu(dv * x) == dv * relu(x) since dv >= 0
                    hn = wrk.tile([P, H], F32, tag="hn")
                    nc.scalar.activation(hn[:, :], ps[:, :], AF.Relu,
                                         scale=dv_t[:, vb:vb + 1])
                    if li > 0:
                        hp = wrk.tile([P, H], F32, tag="hp")
                        nc.scalar.dma_start(hp[:, :], h_d[vb * P:(vb + 1) * P, :])
                        nc.vector.tensor_add(hn[:, :], hn[:, :], hp[:, :])
                    if li < L - 1:
                        nc.scalar.dma_start(h_d[vb * P:(vb + 1) * P, :], hn[:, :])

                    # tail: t = relu(LN_lnxt(hn))
                    st6 = stat.tile([P, 6], F32, tag="st6")
                    nc.vector.bn_stats(st6[:, :], hn[:, :])
                    mv = stat.tile([P, 2], F32, tag="mv")
                    nc.vector.bn_aggr(mv[:, :], st6[:, :])
                    rstd = stat.tile([P, 1], F32, tag="rstd")
                    nc.scalar.activation(rstd[:, :], mv[:, 1:2], AF.Sqrt,
                                         bias=epsc[:, :], scale=1.0)
                    rinv = stat.tile([P, 1], F32, tag="rinv")
                    nc.vector.reciprocal(rinv[:, :], rstd[:, :])
                    nmu = stat.tile([P, 1], F32, tag="nmu")
                    nc.vector.tensor_tensor(nmu[:, :], mv[:, 0:1], rinv[:, :],
                                            op=ALU.mult)
                    zz = wrk.tile([P, H], F32, tag="zz")
                    nc.scalar.activation(zz[:, :], hn[:, :], AF.Identity,
                                         scale=rinv[:, :])
                    nc.vector.tensor_scalar(zz[:, :], zz[:, :], nmu[:, :], None,
                                            op0=ALU.subtract)
                    nc.vector.tensor_tensor(zz[:, :], zz[:, :], lnG_t[lnxt][:, :],
                                            op=ALU.mult)
                    nc.vector.tensor_tensor(zz[:, :], zz[:, :], lnB_t[lnxt][:, :],
                                            op=ALU.add)
                    tb = opool.tile([P, H], BF16, tag="tb")
                    nc.scalar.activation(tb[:, :], zz[:, :], AF.Relu)
                    nc.sync.dma_start(t_d[vb * P:(vb + 1) * P, :], tb[:, :])

        # ------------------------------------------------------------------
        # Final: out = t @ linW + linB
        # ------------------------------------------------------------------
        for rb in range(NBV):
            tin = wrk.tile([P, H], BF16, tag="tin")
            nc.sync.dma_start(tin[:, :], t_d[rb * P:(rb + 1) * P, :])
            tT = opool.tile([P, KH, P], BF16, tag="tT")
            for k in range(KH):
                pst = psT.tile([P, P], F32, tag="psT")
                nc.tensor.matmul(pst[:, :], lhsT=tin[:, k * P:(k + 1) * P],
                                 rhs=ident[:, :], start=True, stop=True)
                nc.scalar.activation(tT[:, k, :], pst[:, :], AF.Copy)
            ps = psT.tile([P, OUT], F32, tag="psO")
            for k in range(KH):
                nc.tensor.matmul(ps[:, :], lhsT=tT[:, k, :], rhs=linW_t[k][:, :],
                                 start=(k == 0), stop=False)
            nc.tensor.matmul(ps[:, :], lhsT=ones1[:1, :], rhs=linB_t[:1, :],
                             start=False, stop=True)
            ob = opool.tile([P, OUT], F32, tag="finout")
            nc.scalar.activation(ob[:, :], ps[:, :], AF.Copy)
            nc.sync.dma_start(out_d[rb * P:(rb + 1) * P, :], ob[:, :])

    nc.compile()
    return nc


# ----------------------------------------------------------------------------
# Full pipeline: prep + build + run
# ----------------------------------------------------------------------------
def run_full(x, vidx, eidx, encW, encB, thetaW, thetaB, lnG, lnB, linW, linB,
             N, E, C, trace=False, nc_cache=None, **runkw):
    IN_DIM = x.shape[1]
    H = encW.shape[1]
    OUT = linW.shape[1]
    L = thetaW.shape[0]

    p = host_prep(np.asarray(vidx), np.asarray(eidx), N, E, C)
    nc = nc_cache if nc_cache is not None else build_program(p, IN_DIM, H, OUT, L)

    x = np.asarray(x, np.float32)
    NP, NPAD = p.NP, p.NPAD
    in_maps = []
    for c in range(C):
        xs = np.zeros((NPAD, IN_DIM), np.float32)
        xs[p.perm[c]] = x[c * NP:(c + 1) * NP]
        in_maps.append(dict(
            xT=np.ascontiguousarray(xs.T).astype(BF16_NP),
            encW=np.asarray(encW, np.float32),
            encB=np.asarray(encB, np.float32),
            thW=np.asarray(thetaW, np.float32),
            thB=np.asarray(thetaB, np.float32),
            lnG=np.asarray(lnG, np.float32),
            lnB=np.asarray(lnB, np.float32),
            linW=np.asarray(linW, np.float32),
            linB=np.asarray(linB, np.float32),
            idxA=p.idxA_w[c],
            rA=p.rA_m[c],
            idxB=p.idxB_w[c],
            rB=p.rB_m[c],
            dvc=p.dvc[c],
            dec=p.dec,
        ))

    res = run_bass_kernel_spmd(nc, in_maps, core_ids=list(range(C)), trace=trace, **runkw)
    out = np.empty((N, OUT), np.float32)
    for c in range(C):
        out[c * NP:(c + 1) * NP] = res.results[c]["out"][p.perm[c]]
    return out, res, nc, p


# hardcoded problem configuration (nn_DeeperHNN_88295937671288)
_N, _E, _NNZ = 100000, 20000, 800000
_C = 8

_nc_cache = None


def kernel(x, vidx, eidx, encW, encB, thetaW, thetaB, lnG, lnB, linW, linB):
    global _nc_cache
    out, res, nc, p = run_full(
        x, vidx, eidx, encW, encB, thetaW, thetaB, lnG, lnB, linW, linB,
        N=_N, E=_E, C=_C, nc_cache=None,
    )
    _nc_cache = nc
    return out.astype(np.float32)
